# revision 20
# baseline (speedup 1.0000x reference)
"""Transformer block (nn_Block_49744311222996) on 8 TRN2 NeuronCores.

Sharding: core c = 2*b + g handles batch b (4 batches); the 1024 query
rows are split between the two cores of a batch in 64-row interleaved
blocks (core g takes global q-blocks {2j+g}), which makes the causal
visible-tile structure identical on every core (n_vis = 9..16) and the
exp/softmax volume perfectly balanced. Each core computes K/V for ALL
16 heads over the full T=2048 (small duplicated matmul work) so that
attention + output projection + FFN for its 512 rows are fully local:
NO collectives at all.

Attention is head-parallel with unnormalized exp + ones-column
denominator (logits are tiny, no max subtraction). Score matmuls for a
head-pair are packed 2x onto the PE array via tile_position row tiling
(K=64 each). K/Q projection chains for pair hp+1 are emitted BETWEEN
the score groups of pair hp (the PE runs in emission order, so this
fills the exp-wait bubbles), and attV for group g is emitted after the
scores of group g+1 (software pipelining).

Algebraic folds (exact): LN1 gamma/beta folded into Wq/Wk/Wv (+ bias
terms), LN2 gamma/beta folded into lin1, K-projection bias dropped
(softmax shift invariance), proj/V biases applied via K=1 matmul rows.

Compute dtype: bf16 matmuls (fp32 PSUM accumulation), fp32 LN/softmax
pointwise, single-instruction Gelu on ScalarE, LN stats via bn_stats.
"""

import numpy as np
import ml_dtypes

import concourse.mybir as mybir
import concourse.tile as tile
from concourse import bacc
from concourse.bass_utils import run_bass_kernel_spmd

F32 = mybir.dt.float32
F32R = mybir.dt.float32r
BF16 = mybir.dt.bfloat16
AF = mybir.ActivationFunctionType
ALU = mybir.AluOpType

B, T, C = 4, 2048, 1024
H, HS = 16, 64
CUT = 1024
P = 128
NT = T // P          # 16 t-tiles
NCt = C // P         # 8 c-tiles
NHP = 8              # head pairs (16 heads)
EPS = 1e-5
ATT_SCALE = float(C) ** -0.5
NF = 4 * C // P      # 32 f-tiles
SR = 512             # q rows per core
NM = SR // P         # 4 q m-tiles
VW = H * 65          # vaug width per t-tile (16 heads x (64+ones))


def _ln_group_stats(nc, pool, xts, eps_ap):
    """LN stats for a group of [128, 1024] fp32 APs via bn_stats.

    Returns (rstd, nmean) [128, len(xts)] fp32 tiles.
    """
    n = len(xts)
    mv = pool.tile([P, n, 2], F32, tag="mv")
    for i, xt in enumerate(xts):
        st = pool.tile([P, 2, 6], F32, tag="bst")
        xr = xt.rearrange("p (s f) -> p s f", f=512)
        for s in range(2):
            nc.vector.bn_stats(st[:, s, :], xr[:, s, :])
        nc.vector.bn_aggr(mv[:, i, :], st[:])
    sd = pool.tile([P, n], F32, tag="sd")
    nc.scalar.activation(sd[:], mv[:, :, 1], AF.Sqrt, bias=eps_ap)
    rstd = pool.tile([P, n], F32, tag="rstd")
    nc.vector.reciprocal(rstd[:], sd[:])
    nmean = pool.tile([P, n], F32, tag="nmean")
    nc.vector.scalar_tensor_tensor(
        out=nmean[:], in0=mv[:, :, 0], scalar=-1.0, in1=rstd[:],
        op0=ALU.mult, op1=ALU.mult,
    )
    return rstd, nmean


def build_nc():
    nc = bacc.Bacc(None, target_bir_lowering=False)

    x = nc.declare_dram_parameter("x", [T, C], F32, isOutput=False)
    xq = nc.declare_dram_parameter("xq", [SR, C], F32, isOutput=False)
    wq = nc.declare_dram_parameter("wq", [C, C], BF16, isOutput=False)
    wk = nc.declare_dram_parameter("wk", [C, C], BF16, isOutput=False)
    wv = nc.declare_dram_parameter("wv", [C, C], BF16, isOutput=False)
    bq = nc.declare_dram_parameter("bq", [P, NHP], F32, isOutput=False)
    bv_row = nc.declare_dram_parameter("bv_row", [1, C], BF16, isOutput=False)
    wproj = nc.declare_dram_parameter("wproj", [C, C], BF16, isOutput=False)
    projb = nc.declare_dram_parameter("projb", [1, C], BF16, isOutput=False)
    lin1 = nc.declare_dram_parameter("lin1", [C, 4 * C], BF16, isOutput=False)
    blin1 = nc.declare_dram_parameter("blin1", [P, NF], F32, isOutput=False)
    lin2 = nc.declare_dram_parameter("lin2", [4 * C, C], BF16, isOutput=False)
    blin2_bc = nc.declare_dram_parameter("blin2_bc", [P, C], F32,
                                         isOutput=False)
    ident = nc.declare_dram_parameter("ident", [P, P], BF16, isOutput=False)
    masks = nc.declare_dram_parameter("masks", [P, 1024], BF16, isOutput=False)
    out = nc.declare_dram_parameter("out", [SR, C], F32, isOutput=True)

    x_tiles = x.rearrange("(n p) c -> n p c", p=P)
    xq_tiles = xq.rearrange("(n p) c -> n p c", p=P)
    out_tiles = out.rearrange("(n p) c -> n p c", p=P)

    with tile.TileContext(nc) as tc:
        with (
            tc.tile_pool(name="const", bufs=1) as const,
            tc.tile_pool(name="stat", bufs=3) as stat,
            tc.tile_pool(name="wB", bufs=3) as wB,    # [128,1024] bf16 h tiles
        ):
            id_sb = const.tile([P, P], BF16)
            nc.sync.dma_start(id_sb[:], ident[:])
            mask_sb = const.tile([P, 1024], BF16)
            nc.sync.dma_start(mask_sb[:], masks[:])
            bq_sb = const.tile([P, NHP], F32)
            nc.sync.dma_start(bq_sb[:], bq[:])
            bv_sb = const.tile([1, C], BF16)
            nc.sync.dma_start(bv_sb[:], bv_row[:])
            projb_sb = const.tile([1, C], BF16)
            nc.sync.dma_start(projb_sb[:], projb[:])
            blin1_sb = const.tile([P, NF], F32)
            nc.sync.dma_start(blin1_sb[:], blin1[:])
            blin2_sb = const.tile([P, C], F32)
            nc.sync.dma_start(blin2_sb[:], blin2_bc[:])
            ones_f = const.tile([1, HS], F32)
            nc.vector.memset(ones_f[:], 1.0)
            ones64 = const.tile([1, HS], F32R)
            with nc.allow_low_precision(reason="f32r ones for bcast matmul"):
                nc.vector.reciprocal(ones64[:], ones_f[:])
            onescol = const.tile([1, P], BF16)
            nc.vector.memset(onescol[:], 1.0)
            eps_sb = const.tile([P, 1], F32)
            nc.vector.memset(eps_sb[:], EPS)

            # persistent across the whole kernel
            res = const.tile([P, NM * C], F32)      # xq, then residual
            oT = const.tile([P, NHP * SR], BF16)    # per-pair o^T blocks
            wp_sb = const.tile([P, NHP * C], BF16)  # proj weights

            with tc.tile_pool(name="abig", bufs=1) as abig:
                hT = abig.tile([P, NCt * T], BF16)    # h^T (c-tile j at j*T)
                hqT = abig.tile([P, NCt * SR], BF16)  # hq^T (my q rows)
                vaug = abig.tile([P, NT * VW], BF16)  # V+ones per t-tile

                # ones columns of vaug (col 64 of each head block)
                va4 = vaug[:].rearrange("p (t h e) -> p t h e", h=H, e=65)
                nc.vector.memset(va4[:, :, :, 64:65], 1.0)

                # ======= LN1 over full T -> hT; V proj per tile =======
                with (
                    tc.tile_pool(name="wv_p", bufs=1) as wv_p,
                    tc.tile_pool(name="wA", bufs=5) as wA,
                    tc.tile_pool(name="pT", bufs=2, space="PSUM") as pT,
                    tc.tile_pool(name="pV", bufs=2, space="PSUM") as pV,
                ):
                    # hq: LN1 on my q rows (same math as the full pass,
                    # duplicated on the per-core xq copy) -> hqT
                    for m in range(NM):
                        nc.sync.dma_start(
                            res[:, m * C:(m + 1) * C], xq_tiles[m]
                        )
                    rstd, nmean = _ln_group_stats(
                        nc, stat,
                        [res[:, m * C:(m + 1) * C] for m in range(NM)],
                        eps_sb[:],
                    )
                    for m in range(NM):
                        hqm = wB.tile([P, C], BF16, tag="ht")
                        nc.scalar.activation(
                            hqm[:], res[:, m * C:(m + 1) * C], AF.Identity,
                            bias=nmean[:, m:m + 1], scale=rstd[:, m:m + 1],
                        )
                        tp = pT.tile([P, C], BF16, tag="tp")
                        for j in range(NCt):
                            nc.tensor.transpose(
                                tp[:, j * P:(j + 1) * P],
                                hqm[:, j * P:(j + 1) * P], id_sb[:]
                            )
                        hqTm = hqT[:].rearrange(
                            "p (j s) -> p j s", j=NCt
                        )[:, :, m * P:(m + 1) * P]
                        nc.vector.tensor_copy(
                            hqTm,
                            tp[:].rearrange("p (j q) -> p j q", j=NCt),
                        )
                    wv_sb = wv_p.tile([P, NCt * C], BF16)
                    wv_t = wv.rearrange("(n p) e -> n p e", p=P)
                    for j in range(NCt):
                        nc.sync.dma_start(
                            wv_sb[:, j * C:(j + 1) * C], wv_t[j]
                        )

                    for grp in range(NT // 4):
                        xts = []
                        for i4 in range(4):
                            xt = wA.tile([P, C], F32, tag="xt")
                            nc.sync.dma_start(xt[:], x_tiles[grp * 4 + i4])
                            xts.append(xt)
                        rstd, nmean = _ln_group_stats(
                            nc, stat, [t[:] for t in xts], eps_sb[:],
                        )
                        for i4 in range(4):
                            i = grp * 4 + i4
                            ht = wB.tile([P, C], BF16, tag="ht")
                            nc.scalar.activation(
                                ht[:], xts[i4][:], AF.Identity,
                                bias=nmean[:, i4:i4 + 1],
                                scale=rstd[:, i4:i4 + 1],
                            )
                            tp = pT.tile([P, C], BF16, tag="tp")
                            for j in range(NCt):
                                nc.tensor.transpose(
                                    tp[:, j * P:(j + 1) * P],
                                    ht[:, j * P:(j + 1) * P], id_sb[:]
                                )
                            hTi = hT[:].rearrange(
                                "p (j t) -> p j t", j=NCt
                            )[:, :, i * P:(i + 1) * P]
                            nc.vector.tensor_copy(
                                hTi,
                                tp[:].rearrange("p (j q) -> p j q", j=NCt),
                            )
                            # V projection for this t-tile (16 heads)
                            for eh in range(2):
                                ps = pV.tile([P, 512], F32, tag="vps")
                                for j in range(NCt):
                                    nc.tensor.matmul(
                                        ps[:],
                                        hT[:, j * T + i * P:
                                           j * T + (i + 1) * P],
                                        wv_sb[:, j * C + eh * 512:
                                              j * C + (eh + 1) * 512],
                                        start=(j == 0), stop=False,
                                    )
                                nc.tensor.matmul(
                                    ps[:], onescol[:],
                                    bv_sb[:, eh * 512:(eh + 1) * 512],
                                    start=False, stop=True,
                                )
                                va = vaug[:, i * VW + eh * 8 * 65:
                                          i * VW + (eh + 1) * 8 * 65]
                                nc.scalar.activation(
                                    va.rearrange("p (h e) -> p h e", e=65)
                                    [:, :, 0:64],
                                    ps[:].rearrange("p (h e) -> p h e", e=64),
                                    AF.Copy,
                                )

                # proj weights: needed at proj time; stream during attention
                wp_t = wproj.rearrange("(n p) c -> n p c", p=P)
                for hp in range(NHP):
                    nc.sync.dma_start(
                        wp_sb[:, hp * C:(hp + 1) * C], wp_t[hp]
                    )

                # ============ attention: per head-pair ============
                wk_t = wk.rearrange("(n p) e -> n p e", p=P)
                wq_t = wq.rearrange("(n p) e -> n p e", p=P)
                with (
                    tc.tile_pool(name="wkq", bufs=2) as wkq,
                    tc.tile_pool(name="kTp", bufs=2) as kTp,
                    tc.tile_pool(name="qTp", bufs=2) as qTp,
                    tc.tile_pool(name="ptp", bufs=4) as ptp,
                    tc.tile_pool(name="pS", bufs=2, space="PSUM") as pS,
                    tc.tile_pool(name="pO", bufs=2, space="PSUM") as pO,
                    tc.tile_pool(name="pM", bufs=2, space="PSUM") as pM,
                ):
                    def emit_kq(hp):
                        """K^T/Q^T projection chains for pair hp.

                        Returns list of emit-thunks (5 chains) plus setup.
                        """
                        wk_sb = wkq.tile([P, NCt * P], BF16, tag="wk",
                                         name=f"wk{hp}")
                        wq_sb = wkq.tile([P, NCt * P], BF16, tag="wq",
                                         name=f"wq{hp}")
                        for j in range(NCt):
                            nc.sync.dma_start(
                                wk_sb[:, j * P:(j + 1) * P],
                                wk_t[j][:, hp * P:(hp + 1) * P],
                            )
                            nc.sync.dma_start(
                                wq_sb[:, j * P:(j + 1) * P],
                                wq_t[j][:, hp * P:(hp + 1) * P],
                            )
                        kT = kTp.tile([P, T], BF16, tag="kT",
                                      name=f"kT{hp}")
                        qT = qTp.tile([P, SR], BF16, tag="qT",
                                      name=f"qT{hp}")

                        def k_chain(tch):
                            def f():
                                ps = pM.tile([P, 512], F32, tag="mm",
                                             name=f"kc{hp}_{tch}")
                                for j in range(NCt):
                                    nc.tensor.matmul(
                                        ps[:],
                                        wk_sb[:, j * P:(j + 1) * P],
                                        hT[:, j * T + tch * 512:
                                           j * T + (tch + 1) * 512],
                                        start=(j == 0), stop=(j == NCt - 1),
                                    )
                                nc.vector.tensor_copy(
                                    kT[:, tch * 512:(tch + 1) * 512], ps[:]
                                )
                            return f

                        def q_chain():
                            ps = pM.tile([P, 512], F32, tag="mm",
                                         name=f"qc{hp}")
                            for j in range(NCt):
                                nc.tensor.matmul(
                                    ps[:],
                                    wq_sb[:, j * P:(j + 1) * P],
                                    hqT[:, j * SR:(j + 1) * SR],
                                    start=(j == 0), stop=(j == NCt - 1),
                                )
                            nc.vector.tensor_scalar_add(
                                qT[:], ps[:], bq_sb[:, hp:hp + 1]
                            )

                        thunks = [k_chain(t) for t in range(4)] + [q_chain]
                        return kT, qT, thunks

                    kT, qT, thunks = emit_kq(0)
                    for th in thunks:
                        th()

                    for hp in range(NHP):
                        if hp < NHP - 1:
                            kT_n, qT_n, thunks = emit_kq(hp + 1)
                        else:
                            thunks = []
                        opsA = pO.tile([65, SR], F32, tag="ops",
                                       name=f"opsA{hp}")
                        opsB = pO.tile([65, SR], F32, tag="ops",
                                       name=f"opsB{hp}")
                        ti = 0
                        for ch in range(2):
                            n_vis = 12 + 4 * ch
                            prev = None
                            for g in range(n_vis // 4):
                                sA = pS.tile([P, 1024], F32, tag="sc",
                                             name=f"sA{hp}_{ch}_{g}")
                                sB = pS.tile([P, 1024], F32, tag="sc",
                                             name=f"sB{hp}_{ch}_{g}")
                                for t4 in range(4):
                                    tt = g * 4 + t4
                                    nc.tensor.matmul(
                                        sA[:, t4 * 256:(t4 + 1) * 256],
                                        kT[0:64, tt * P:(tt + 1) * P],
                                        qT[0:64, ch * 256:(ch + 1) * 256],
                                        start=True, stop=True,
                                    )
                                    nc.tensor.matmul(
                                        sB[:, t4 * 256:(t4 + 1) * 256],
                                        kT[64:128, tt * P:(tt + 1) * P],
                                        qT[64:128, ch * 256:(ch + 1) * 256],
                                        start=True, stop=True,
                                    )
                                if ti < len(thunks):
                                    thunks[ti]()
                                    ti += 1
                                ptA = ptp.tile([P, 1024], BF16, tag="pt",
                                               name=f"pA{hp}_{ch}_{g}")
                                ptB = ptp.tile([P, 1024], BF16, tag="pt",
                                               name=f"pB{hp}_{ch}_{g}")
                                nc.scalar.activation(
                                    ptA[:], sA[:], AF.Exp, scale=ATT_SCALE
                                )
                                nc.scalar.activation(
                                    ptB[:], sB[:], AF.Exp, scale=ATT_SCALE
                                )
                                if g == 2 + ch:  # boundary group: masks
                                    nc.vector.tensor_mul(
                                        ptA[:], ptA[:], mask_sb[:]
                                    )
                                    nc.vector.tensor_mul(
                                        ptB[:], ptB[:], mask_sb[:]
                                    )

                                def attv(gg, pA, pB):
                                    for t4 in range(4):
                                        tt = gg * 4 + t4
                                        nc.tensor.matmul(
                                            opsA[:, ch * 256:(ch + 1) * 256],
                                            vaug[:, tt * VW + 2 * hp * 65:
                                                 tt * VW + 2 * hp * 65 + 65],
                                            pA[:, t4 * 256:(t4 + 1) * 256],
                                            start=(tt == 0),
                                            stop=(tt == n_vis - 1),
                                        )
                                        nc.tensor.matmul(
                                            opsB[:, ch * 256:(ch + 1) * 256],
                                            vaug[:,
                                                 tt * VW + (2 * hp + 1) * 65:
                                                 tt * VW + (2 * hp + 1) * 65
                                                 + 65],
                                            pB[:, t4 * 256:(t4 + 1) * 256],
                                            start=(tt == 0),
                                            stop=(tt == n_vis - 1),
                                        )

                                if prev is not None:
                                    attv(*prev)
                                prev = (g, ptA, ptB)
                            attv(*prev)

                        # normalize: oT[...] = ops[0:64] / ops[64]
                        for r, ops in ((0, opsA), (1, opsB)):
                            rd = stat.tile([1, SR], F32R, tag="rd")
                            with nc.allow_low_precision(
                                reason="f32r recip for attn denom bcast"
                            ):
                                nc.vector.reciprocal(rd[:], ops[64:65, :])
                            rb = pM.tile([P, 512], F32, tag="mm",
                                         name=f"rb{hp}_{r}")
                            nc.tensor.matmul(
                                rb[0:64, :], ones64[:], rd[:],
                                start=True, stop=True,
                            )
                            rbs = stat.tile([64, SR], F32, tag="rbs")
                            nc.vector.tensor_copy(rbs[:], rb[0:64, :])
                            nc.vector.tensor_mul(
                                oT[64 * r:64 * (r + 1),
                                   hp * SR:(hp + 1) * SR],
                                ops[0:64, :], rbs[:],
                            )
                        if hp < NHP - 1:
                            kT, qT = kT_n, qT_n

            # ============ proj + residual -> res; LN2 -> h2T; FFN =======
            with tc.tile_pool(name="post", bufs=1) as post:
                h2T = post.tile([P, NCt * SR], BF16)
                gT = post.tile([P, NF * SR], BF16)
                with (
                    tc.tile_pool(name="pP2", bufs=3, space="PSUM") as pP2,
                    tc.tile_pool(name="pT2", bufs=2, space="PSUM") as pT2,
                ):
                    for m in range(NM):
                        for nh in range(2):
                            ps = pP2.tile([P, 512], F32, tag="mm")
                            for hp in range(NHP):
                                nc.tensor.matmul(
                                    ps[:],
                                    oT[:, hp * SR + m * P:
                                       hp * SR + (m + 1) * P],
                                    wp_sb[:, hp * C + nh * 512:
                                          hp * C + (nh + 1) * 512],
                                    start=(hp == 0), stop=False,
                                )
                            # proj bias via K=1 matmul (ones x bias row)
                            nc.tensor.matmul(
                                ps[:], onescol[:],
                                projb_sb[:, nh * 512:(nh + 1) * 512],
                                start=False, stop=True,
                            )
                            rm = res[:, m * C + nh * 512:
                                     m * C + (nh + 1) * 512]
                            nc.vector.tensor_add(rm, rm, ps[:])
                    rstd, nmean = _ln_group_stats(
                        nc, stat,
                        [res[:, m * C:(m + 1) * C] for m in range(NM)],
                        eps_sb[:],
                    )
                    for m in range(NM):
                        h2 = wB.tile([P, C], BF16, tag="ht")
                        nc.scalar.activation(
                            h2[:], res[:, m * C:(m + 1) * C], AF.Identity,
                            bias=nmean[:, m:m + 1], scale=rstd[:, m:m + 1],
                        )
                        tp = pT2.tile([P, C], BF16, tag="tp")
                        for j in range(NCt):
                            nc.tensor.transpose(
                                tp[:, j * P:(j + 1) * P],
                                h2[:, j * P:(j + 1) * P], id_sb[:]
                            )
                        h2Tm = h2T[:].rearrange(
                            "p (j s) -> p j s", j=NCt
                        )[:, :, m * P:(m + 1) * P]
                        nc.vector.tensor_copy(
                            h2Tm,
                            tp[:].rearrange("p (j q) -> p j q", j=NCt),
                        )
                        # fold lin2 bias into res for the final add
                        nc.vector.tensor_add(
                            res[:, m * C:(m + 1) * C],
                            res[:, m * C:(m + 1) * C], blin2_sb[:],
                        )

                # ============ FFN1 (gelu) -> gT ============
                l1_t = lin1.rearrange("(n p) f -> n p f", p=P)
                with (
                    tc.tile_pool(name="ffB", bufs=2) as ffB,
                    tc.tile_pool(name="pG", bufs=2, space="PSUM") as pG,
                ):
                    for quar in range(4):
                        l1h = ffB.tile([P, NCt * C], BF16, tag="l1")
                        for j in range(NCt):
                            nc.sync.dma_start(
                                l1h[:, j * C:(j + 1) * C],
                                l1_t[j][:, quar * C:(quar + 1) * C],
                            )
                        for fl in range(NF // 4):
                            ft = quar * (NF // 4) + fl
                            ps = pG.tile([P, 512], F32, tag="mm")
                            for j in range(NCt):
                                nc.tensor.matmul(
                                    ps[:],
                                    l1h[:, j * C + fl * P:
                                        j * C + (fl + 1) * P],
                                    h2T[:, j * SR:(j + 1) * SR],
                                    start=(j == 0), stop=(j == NCt - 1),
                                )
                            nc.scalar.activation(
                                gT[:, ft * SR:(ft + 1) * SR], ps[:], AF.Gelu,
                                bias=blin1_sb[:, ft:ft + 1],
                            )

                # ============ FFN2 + residual -> out ============
                # lin2 resident in SBUF; one PSUM bank per (nh, m) chain
                # accumulated ft-contiguously (bank cycling per-MM makes
                # the HAM clock oscillate -- FFN1-style chains stay warm).
                l2_t = lin2.rearrange("(n p) c -> n p c", p=P)
                l2_sb = post.tile([P, NF * C], BF16)
                for ft in range(NF):
                    nc.sync.dma_start(
                        l2_sb[:, ft * C:(ft + 1) * C], l2_t[ft]
                    )
                with (
                    tc.tile_pool(name="ffW", bufs=3) as ffW,
                    tc.tile_pool(name="pF", bufs=2, space="PSUM") as pF,
                ):
                    for nh in range(2):
                        for m in range(NM):
                            fpt = pF.tile([P, 512], F32, tag="ff")
                            for ft in range(NF):
                                nc.tensor.matmul(
                                    fpt[:],
                                    gT[:, ft * SR + m * P:
                                       ft * SR + (m + 1) * P],
                                    l2_sb[:, ft * C + nh * 512:
                                          ft * C + (nh + 1) * 512],
                                    start=(ft == 0), stop=(ft == NF - 1),
                                )
                            o_sb = ffW.tile([P, 512], F32, tag="osb")
                            nc.vector.tensor_add(
                                o_sb[:], fpt[:],
                                res[:, m * C + nh * 512:
                                    m * C + (nh + 1) * 512],
                            )
                            nc.sync.dma_start(
                                out_tiles[m][:, nh * 512:(nh + 1) * 512],
                                o_sb[:],
                            )

    nc.compile()
    return nc


_NC = None


def _get_nc():
    global _NC
    if _NC is None:
        _NC = build_nc()
    return _NC


def kernel(**inputs):
    nc = _get_nc()
    bf = ml_dtypes.bfloat16
    f32 = np.float32

    x = np.asarray(inputs["x"], f32)
    Wq = np.asarray(inputs["Wq"], f32)
    Wk = np.asarray(inputs["Wk"], f32)
    Wv = np.asarray(inputs["Wv"], f32)
    bq = np.asarray(inputs["bq"], f32)
    bv = np.asarray(inputs["bv"], f32)
    proj_w = np.asarray(inputs["proj_w"], f32)
    proj_b = np.asarray(inputs["proj_b"], f32)
    ln1_w = np.asarray(inputs["ln1_w"], f32)
    ln1_b = np.asarray(inputs["ln1_b"], f32)
    ln2_w = np.asarray(inputs["ln2_w"], f32)
    ln2_b = np.asarray(inputs["ln2_b"], f32)
    lin1_w = np.asarray(inputs["lin1_w"], f32)
    lin1_b = np.asarray(inputs["lin1_b"], f32)
    lin2_w = np.asarray(inputs["lin2_w"], f32)
    lin2_b = np.asarray(inputs["lin2_b"], f32)

    # LN1 gamma folded into QKV weights; beta folded into biases.
    Wq_f = Wq * ln1_w[None, :, None]              # [H, C, HS]
    Wk_f = Wk * ln1_w[None, :, None]
    Wv_f = Wv * ln1_w[None, :, None]
    bq_eff = bq + np.einsum("c,hcd->hd", ln1_b, Wq)   # [H, HS]
    bv_eff = bv + np.einsum("c,hcd->hd", ln1_b, Wv)
    # (bk dropped: softmax shift invariance)
    wq_full = np.ascontiguousarray(
        Wq_f.transpose(1, 0, 2).reshape(C, C)).astype(bf)
    wk_full = np.ascontiguousarray(
        Wk_f.transpose(1, 0, 2).reshape(C, C)).astype(bf)
    wv_full = np.ascontiguousarray(
        Wv_f.transpose(1, 0, 2).reshape(C, C)).astype(bf)
    bq_t = np.ascontiguousarray(bq_eff.reshape(NHP, P).T).astype(f32)
    bv_rowh = bv_eff.reshape(1, C).astype(bf)

    # LN2 gamma folded into lin1; beta into its bias.
    lin1_f = (lin1_w * ln2_w[:, None]).astype(bf)
    blin1_eff = lin1_b + ln2_b @ lin1_w
    blin1_t = np.ascontiguousarray(blin1_eff.reshape(NF, P).T).astype(f32)
    lin2_bf = lin2_w.astype(bf)
    blin2_bc = np.ascontiguousarray(
        np.broadcast_to(lin2_b, (P, C))).astype(f32)

    proj_bf = proj_w.astype(bf)
    projb_row = proj_b.reshape(1, C).astype(bf)
    ident = np.eye(P, dtype=bf)

    in_maps = []
    for c in range(8):
        b, g = divmod(c, 2)
        # my q rows: 64-row blocks {2j+g}, local row 64j+r
        j = np.arange(NCt)
        r = np.arange(64)
        rows = (T - CUT) + 64 * (2 * j[:, None] + g) + r[None, :]
        rows = rows.reshape(-1)
        xq_c = np.ascontiguousarray(x[b][rows]).astype(f32)
        # masks: [128, 4*256]: block i, col 64*jj+rr:
        #   visible iff toff <= 128*jj + 64*g + rr - 128*i
        toff = np.arange(P)[:, None]
        i_b = np.arange(4)[:, None, None]
        jj = np.arange(4)[None, :, None]
        rr = np.arange(64)[None, None, :]
        thr = (128 * jj + 64 * g + rr - 128 * i_b).reshape(1, 1024)
        mask_c = (toff <= thr).astype(bf)
        in_maps.append({
            "x": np.ascontiguousarray(x[b]),
            "xq": xq_c,
            "wq": wq_full, "wk": wk_full, "wv": wv_full,
            "bq": bq_t, "bv_row": bv_rowh,
            "wproj": proj_bf, "projb": projb_row,
            "lin1": lin1_f, "blin1": blin1_t,
            "lin2": lin2_bf, "blin2_bc": blin2_bc,
            "ident": ident, "masks": np.ascontiguousarray(mask_c),
        })

    resl = run_bass_kernel_spmd(nc, in_maps, core_ids=list(range(8)))
    out_full = np.empty((B, CUT, C), f32)
    jj = np.arange(NCt)
    rr = np.arange(64)
    for c in range(8):
        b, g = divmod(c, 2)
        rows = (64 * (2 * jj[:, None] + g) + rr[None, :]).reshape(-1)
        out_full[b, rows, :] = resl.results[c]["out"]
    return out_full


# revision 24
# speedup vs baseline: 1.3056x; 1.3056x over previous
"""Transformer block (nn_Block_49744311222996) on 8 TRN2 NeuronCores.

Sharding: core c = 2*b + g handles batch b (4 batches); the 1024 query
rows are split between the two cores of a batch in 64-row interleaved
blocks (core g takes global q-blocks {2j+g}), which makes the causal
visible-tile structure identical on every core (n_vis = 9..16) and the
exp/softmax volume perfectly balanced. Each core computes K/V for ALL
16 heads over the full T=2048 so attention + output projection + FFN
for its 512 rows are fully local: NO collectives.

fp8(e4m3) + DoubleRow (K=256 per matmul) for the whole attention half:
h/hq are stored fp8; Wq/Wk/Wv/Wproj are host-scaled by 32 and cast to
fp8 (weights at 0.02 scale would be subnormal in e4m3); the resulting
power-of-two scale factors are folded into the exp() scale and a final
1/1024 on the projection output. V and the attention probabilities are
fp8 as well (attV in DoubleRow). Scores stay bf16 (DoubleRow cannot
pair across heads), FFN stays bf16 (precision budget).

K/Q projection chains for pair hp+1 are emitted BETWEEN the score
groups of pair hp (the PE runs in emission order, so this fills the
exp-wait bubbles); attV for group g is emitted after the scores of
group g+1 (software pipelining). FFN2 accumulates each PSUM bank
ft-contiguously with lin2 resident in SBUF (per-matmul bank cycling
makes the HAM clock oscillate).

All reciprocals/rsqrt run on ScalarE as exp(-ln x) / exp(-ln(x)/2); the
activation-table choice is pinned to natural_log_exp_and_others so the
whole pre-FFN kernel needs a single table load (plus one for Gelu).

Algebraic folds (exact): LN1 gamma/beta into Wq/Wk/Wv (+ bias terms),
LN2 gamma/beta into lin1, K-projection bias dropped (softmax shift
invariance), proj/V biases via K=1 matmul rows.
"""

import numpy as np
import ml_dtypes

import concourse.mybir as mybir
import concourse.tile as tile
from concourse import bacc
from concourse.bass_utils import run_bass_kernel_spmd

F32 = mybir.dt.float32
F32R = mybir.dt.float32r
BF16 = mybir.dt.bfloat16
F8 = mybir.dt.float8e4
AF = mybir.ActivationFunctionType
ALU = mybir.AluOpType

B, T, C = 4, 2048, 1024
H, HS = 16, 64
CUT = 1024
P = 128
NT = T // P          # 16 t-tiles
NCt = C // P         # 8 c-tiles
NHP = 8              # head pairs (16 heads)
EPS = 1e-5
WS = 32.0            # fp8 weight scale (Wq/Wk/Wv/Wproj x32)
ATT_SCALE = float(C) ** -0.5 / (WS * WS)
NF = 4 * C // P      # 32 f-tiles
SR = 512             # q rows per core
NM = SR // P         # 4 q m-tiles
VW = H * 65          # vaug width per t-tile (16 heads x (64+ones))


def _ln_group_stats(nc, pool, xts, eps_ap):
    """LN stats for a group of [128, 1024] fp32 APs via bn_stats.

    rstd = exp(-ln(var+eps)/2) -- stays in the natural_log_exp table
    set. Returns (rstd, nmean) [128, len(xts)] fp32 tiles.
    """
    n = len(xts)
    mv = pool.tile([P, n, 2], F32, tag="mv")
    for i, xt in enumerate(xts):
        st = pool.tile([P, 2, 6], F32, tag="bst")
        xr = xt.rearrange("p (s f) -> p s f", f=512)
        for s in range(2):
            nc.vector.bn_stats(st[:, s, :], xr[:, s, :])
        nc.vector.bn_aggr(mv[:, i, :], st[:])
    lv = pool.tile([P, n], F32, tag="lv")
    nc.scalar.activation(lv[:], mv[:, :, 1], AF.Ln, bias=eps_ap)
    rstd = pool.tile([P, n], F32, tag="rstd")
    nc.scalar.activation(rstd[:], lv[:], AF.Exp, scale=-0.5)
    nmean = pool.tile([P, n], F32, tag="nmean")
    nc.vector.scalar_tensor_tensor(
        out=nmean[:], in0=mv[:, :, 0], scalar=-1.0, in1=rstd[:],
        op0=ALU.mult, op1=ALU.mult,
    )
    return rstd, nmean


def build_nc():
    # Pin activation-table choice: empty every set except the two we
    # want so the load-insertion pass cannot ping-pong between sets.
    orig_tabs = bacc.get_activation_tables

    def pinned_tabs(arch):
        t = dict(orig_tabs(arch))
        keep = {"natural_log_exp_and_others", "gelu_and_others"}
        assert keep <= set(t), sorted(t)
        need = {AF.Exp, AF.Ln, AF.Identity, AF.Copy}
        assert need <= t["natural_log_exp_and_others"]
        assert AF.Gelu in t["gelu_and_others"]
        return {k: (v if k in keep else set()) for k, v in t.items()}

    bacc.get_activation_tables = pinned_tabs
    try:
        return _build_nc_inner()
    finally:
        bacc.get_activation_tables = orig_tabs


def _build_nc_inner():
    nc = bacc.Bacc(None, target_bir_lowering=False)

    x = nc.declare_dram_parameter("x", [T, C], F32, isOutput=False)
    xq = nc.declare_dram_parameter("xq", [SR, C], F32, isOutput=False)
    wq = nc.declare_dram_parameter("wq", [C, C], F8, isOutput=False)
    wk = nc.declare_dram_parameter("wk", [C, C], F8, isOutput=False)
    wv = nc.declare_dram_parameter("wv", [C, C], F8, isOutput=False)
    bq = nc.declare_dram_parameter("bq", [P, NHP], F32, isOutput=False)
    bv_row = nc.declare_dram_parameter("bv_row", [1, C], BF16, isOutput=False)
    wproj = nc.declare_dram_parameter("wproj", [C, C], F8, isOutput=False)
    projb = nc.declare_dram_parameter("projb", [1, C], BF16, isOutput=False)
    lin1 = nc.declare_dram_parameter("lin1", [C, 4 * C], BF16, isOutput=False)
    blin1 = nc.declare_dram_parameter("blin1", [P, NF], F32, isOutput=False)
    lin2 = nc.declare_dram_parameter("lin2", [4 * C, C], BF16, isOutput=False)
    blin2_bc = nc.declare_dram_parameter("blin2_bc", [P, C], F32,
                                         isOutput=False)
    ident = nc.declare_dram_parameter("ident", [P, P], BF16, isOutput=False)
    ident8 = nc.declare_dram_parameter("ident8", [P, P], F8, isOutput=False)
    masks = nc.declare_dram_parameter("masks", [P, 1024], F8, isOutput=False)
    out = nc.declare_dram_parameter("out", [SR, C], F32, isOutput=True)

    x_tiles = x.rearrange("(n p) c -> n p c", p=P)
    xq_tiles = xq.rearrange("(n p) c -> n p c", p=P)
    out_tiles = out.rearrange("(n p) c -> n p c", p=P)

    with tile.TileContext(nc) as tc:
        with (
            tc.tile_pool(name="const", bufs=1) as const,
            tc.tile_pool(name="stat", bufs=3) as stat,
            tc.tile_pool(name="wB", bufs=3) as wB,
        ):
            id_sb = const.tile([P, P], BF16)
            nc.sync.dma_start(id_sb[:], ident[:])
            id8_sb = const.tile([P, P], F8)
            nc.sync.dma_start(id8_sb[:], ident8[:])
            mask_sb = const.tile([P, 1024], F8)
            nc.sync.dma_start(mask_sb[:], masks[:])
            bq_sb = const.tile([P, NHP], F32)
            nc.sync.dma_start(bq_sb[:], bq[:])
            bv_sb = const.tile([1, C], BF16)
            nc.sync.dma_start(bv_sb[:], bv_row[:])
            projb_sb = const.tile([1, C], BF16)
            nc.sync.dma_start(projb_sb[:], projb[:])
            blin1_sb = const.tile([P, NF], F32)
            nc.sync.dma_start(blin1_sb[:], blin1[:])
            blin2_sb = const.tile([P, C], F32)
            nc.sync.dma_start(blin2_sb[:], blin2_bc[:])
            ones_f = const.tile([1, HS], F32)
            nc.vector.memset(ones_f[:], 1.0)
            ones64 = const.tile([1, HS], F32R)
            with nc.allow_low_precision(reason="f32r ones for bcast matmul"):
                nc.vector.reciprocal(ones64[:], ones_f[:])
            onescol = const.tile([1, P], BF16)
            nc.vector.memset(onescol[:], 1.0)
            eps_sb = const.tile([P, 1], F32)
            nc.vector.memset(eps_sb[:], EPS)

            # persistent across the whole kernel
            res = const.tile([P, NM * C], F32)      # xq, then residual
            oT = const.tile([P, NHP * SR], F8)      # per-pair o^T blocks
            wp_sb = const.tile([P, NHP * C], F8)    # proj weights (x32 fp8)

            with tc.tile_pool(name="abig", bufs=1) as abig:
                hT = abig.tile([P, NCt * T], F8)     # h^T (c-tile j at j*T)
                hqT = abig.tile([P, NCt * SR], F8)   # hq^T (my q rows)
                vaug = abig.tile([P, NT * VW], F8)   # V+ones per t-tile
                hTj = hT[:].rearrange("p (j t) -> p j t", j=NCt)
                hqTj = hqT[:].rearrange("p (j s) -> p j s", j=NCt)
                vaug4 = vaug[:].rearrange("p (t h e) -> p t h e", h=H, e=65)
                nc.vector.memset(vaug4[:, :, :, 64:65], 1.0)

                # ======= LN1; hq first (q rows), then full T + V proj ====
                with (
                    tc.tile_pool(name="wv_p", bufs=1) as wv_p,
                    tc.tile_pool(name="wA", bufs=5) as wA,
                    tc.tile_pool(name="pT", bufs=2, space="PSUM") as pT,
                    tc.tile_pool(name="pV", bufs=2, space="PSUM") as pV,
                ):
                    for m in range(NM):
                        nc.sync.dma_start(
                            res[:, m * C:(m + 1) * C], xq_tiles[m]
                        )
                    rstd, nmean = _ln_group_stats(
                        nc, stat,
                        [res[:, m * C:(m + 1) * C] for m in range(NM)],
                        eps_sb[:],
                    )
                    for m in range(NM):
                        hqm = wB.tile([P, C], BF16, tag="ht")
                        nc.scalar.activation(
                            hqm[:], res[:, m * C:(m + 1) * C], AF.Identity,
                            bias=nmean[:, m:m + 1], scale=rstd[:, m:m + 1],
                        )
                        tp = pT.tile([P, C], BF16, tag="tp")
                        for j in range(NCt):
                            nc.tensor.transpose(
                                tp[:, j * P:(j + 1) * P],
                                hqm[:, j * P:(j + 1) * P], id_sb[:]
                            )
                        nc.vector.tensor_copy(
                            hqTj[:, :, m * P:(m + 1) * P],
                            tp[:].rearrange("p (j q) -> p j q", j=NCt),
                        )

                    wv_sb = wv_p.tile([P, NCt * C], F8)
                    wvj = wv_sb[:].rearrange("p (j e) -> p j e", j=NCt)
                    wv_t = wv.rearrange("(n p) e -> n p e", p=P)
                    for j in range(NCt):
                        nc.sync.dma_start(
                            wv_sb[:, j * C:(j + 1) * C], wv_t[j]
                        )

                    for grp in range(NT // 4):
                        xts = []
                        for i4 in range(4):
                            xt = wA.tile([P, C], F32, tag="xt")
                            nc.sync.dma_start(xt[:], x_tiles[grp * 4 + i4])
                            xts.append(xt)
                        rstd, nmean = _ln_group_stats(
                            nc, stat, [t[:] for t in xts], eps_sb[:],
                        )
                        for i4 in range(4):
                            i = grp * 4 + i4
                            ht = wB.tile([P, C], BF16, tag="ht")
                            nc.scalar.activation(
                                ht[:], xts[i4][:], AF.Identity,
                                bias=nmean[:, i4:i4 + 1],
                                scale=rstd[:, i4:i4 + 1],
                            )
                            tp = pT.tile([P, C], BF16, tag="tp")
                            for j in range(NCt):
                                nc.tensor.transpose(
                                    tp[:, j * P:(j + 1) * P],
                                    ht[:, j * P:(j + 1) * P], id_sb[:]
                                )
                            nc.vector.tensor_copy(
                                hTj[:, :, i * P:(i + 1) * P],
                                tp[:].rearrange("p (j q) -> p j q", j=NCt),
                            )
                            # V projection for this t-tile (all 16 heads),
                            # fp8 DoubleRow over c-tile pairs
                            ps = pV.tile([P, 1024], F32, tag="vps")
                            for q4 in range(4):
                                for jj in range(4):
                                    nc.tensor.matmul(
                                        ps[:, q4 * 256:(q4 + 1) * 256],
                                        hTj[:, 2 * jj:2 * jj + 2,
                                            i * P:(i + 1) * P],
                                        wvj[:, 2 * jj:2 * jj + 2,
                                            q4 * 256:(q4 + 1) * 256],
                                        start=(jj == 0), stop=False,
                                        perf_mode=(
                                            mybir.MatmulPerfMode.DoubleRow),
                                    )
                            for eh in range(2):
                                nc.tensor.matmul(
                                    ps[:, eh * 512:(eh + 1) * 512],
                                    onescol[:],
                                    bv_sb[:, eh * 512:(eh + 1) * 512],
                                    start=False, stop=True,
                                    skip_group_check=True,
                                )
                            nc.scalar.activation(
                                vaug4[:, i, :, 0:64],
                                ps[:].rearrange("p (h e) -> p h e", e=64),
                                AF.Copy,
                            )

                # proj weights: needed at proj time; stream during attention
                wp_t = wproj.rearrange("(n p) c -> n p c", p=P)
                for hp in range(NHP):
                    nc.sync.dma_start(
                        wp_sb[:, hp * C:(hp + 1) * C], wp_t[hp]
                    )

                # ============ attention: per head-pair ============
                wk_t = wk.rearrange("(n p) e -> n p e", p=P)
                wq_t = wq.rearrange("(n p) e -> n p e", p=P)
                with (
                    tc.tile_pool(name="wkq", bufs=2) as wkq,
                    tc.tile_pool(name="kTp", bufs=2) as kTp,
                    tc.tile_pool(name="qTp", bufs=2) as qTp,
                    tc.tile_pool(name="ptp", bufs=4) as ptp,
                    tc.tile_pool(name="pS", bufs=2, space="PSUM") as pS,
                    tc.tile_pool(name="pO", bufs=2, space="PSUM") as pO,
                    tc.tile_pool(name="pM", bufs=2, space="PSUM") as pM,
                ):
                    DR = mybir.MatmulPerfMode.DoubleRow

                    def emit_kq(hp):
                        wk_sb = wkq.tile([P, NCt * P], F8, tag="wk",
                                         name=f"wk{hp}")
                        wq_sb = wkq.tile([P, NCt * P], F8, tag="wq",
                                         name=f"wq{hp}")
                        for j in range(NCt):
                            nc.sync.dma_start(
                                wk_sb[:, j * P:(j + 1) * P],
                                wk_t[j][:, hp * P:(hp + 1) * P],
                            )
                            nc.sync.dma_start(
                                wq_sb[:, j * P:(j + 1) * P],
                                wq_t[j][:, hp * P:(hp + 1) * P],
                            )
                        wkp = wk_sb[:].rearrange("p (j e) -> p j e", j=NCt)
                        wqp = wq_sb[:].rearrange("p (j e) -> p j e", j=NCt)
                        kT = kTp.tile([P, T], BF16, tag="kT", name=f"kT{hp}")
                        qT = qTp.tile([P, SR], BF16, tag="qT", name=f"qT{hp}")

                        def k_chain(tch):
                            def f():
                                ps = pM.tile([P, 512], F32, tag="mm",
                                             name=f"kc{hp}_{tch}")
                                for n2 in range(2):
                                    for jj in range(4):
                                        nc.tensor.matmul(
                                            ps[:, n2 * 256:(n2 + 1) * 256],
                                            wkp[:, 2 * jj:2 * jj + 2, :],
                                            hTj[:, 2 * jj:2 * jj + 2,
                                                tch * 512 + n2 * 256:
                                                tch * 512 + (n2 + 1) * 256],
                                            start=(jj == 0), stop=(jj == 3),
                                            perf_mode=DR,
                                        )
                                nc.vector.tensor_copy(
                                    kT[:, tch * 512:(tch + 1) * 512], ps[:]
                                )
                            return f

                        def q_chain():
                            ps = pM.tile([P, 512], F32, tag="mm",
                                         name=f"qc{hp}")
                            for n2 in range(2):
                                for jj in range(4):
                                    nc.tensor.matmul(
                                        ps[:, n2 * 256:(n2 + 1) * 256],
                                        wqp[:, 2 * jj:2 * jj + 2, :],
                                        hqTj[:, 2 * jj:2 * jj + 2,
                                             n2 * 256:(n2 + 1) * 256],
                                        start=(jj == 0), stop=(jj == 3),
                                        perf_mode=DR,
                                    )
                            nc.vector.tensor_scalar_add(
                                qT[:], ps[:], bq_sb[:, hp:hp + 1]
                            )

                        thunks = [k_chain(t) for t in range(4)] + [q_chain]
                        return kT, qT, thunks

                    kT, qT, thunks = emit_kq(0)
                    for th in thunks:
                        th()

                    for hp in range(NHP):
                        if hp < NHP - 1:
                            kT_n, qT_n, thunks = emit_kq(hp + 1)
                        else:
                            thunks = []
                        opsA = pO.tile([65, SR], F32, tag="ops",
                                       name=f"opsA{hp}")
                        opsB = pO.tile([65, SR], F32, tag="ops",
                                       name=f"opsB{hp}")
                        ti = 0
                        for ch in range(2):
                            n_vis = 12 + 4 * ch
                            prev = None
                            for g in range(n_vis // 4):
                                sA = pS.tile([P, 1024], F32, tag="sc",
                                             name=f"sA{hp}_{ch}_{g}")
                                sB = pS.tile([P, 1024], F32, tag="sc",
                                             name=f"sB{hp}_{ch}_{g}")
                                for t4 in range(4):
                                    tt = g * 4 + t4
                                    nc.tensor.matmul(
                                        sA[:, t4 * 256:(t4 + 1) * 256],
                                        kT[0:64, tt * P:(tt + 1) * P],
                                        qT[0:64, ch * 256:(ch + 1) * 256],
                                        start=True, stop=True,
                                    )
                                    nc.tensor.matmul(
                                        sB[:, t4 * 256:(t4 + 1) * 256],
                                        kT[64:128, tt * P:(tt + 1) * P],
                                        qT[64:128, ch * 256:(ch + 1) * 256],
                                        start=True, stop=True,
                                    )
                                if ti < len(thunks):
                                    thunks[ti]()
                                    ti += 1
                                ptA = ptp.tile([P, 1024], F8, tag="pt",
                                               name=f"pA{hp}_{ch}_{g}")
                                ptB = ptp.tile([P, 1024], F8, tag="pt",
                                               name=f"pB{hp}_{ch}_{g}")
                                nc.scalar.activation(
                                    ptA[:], sA[:], AF.Exp, scale=ATT_SCALE
                                )
                                nc.scalar.activation(
                                    ptB[:], sB[:], AF.Exp, scale=ATT_SCALE
                                )
                                if g == 2 + ch:  # boundary group: masks
                                    nc.vector.tensor_mul(
                                        ptA[:], ptA[:], mask_sb[:]
                                    )
                                    nc.vector.tensor_mul(
                                        ptB[:], ptB[:], mask_sb[:]
                                    )

                                def attv(gg, pA, pB):
                                    for t2 in range(2):
                                        tt = gg * 4 + 2 * t2
                                        for r, ops, pp in (
                                            (0, opsA, pA), (1, opsB, pB)
                                        ):
                                            nc.tensor.matmul(
                                                ops[:,
                                                    ch * 256:(ch + 1) * 256],
                                                vaug4[:, tt:tt + 2,
                                                      2 * hp + r, :],
                                                pp[:, 2 * t2 * 256:
                                                   2 * (t2 + 1) * 256]
                                                .rearrange(
                                                    "p (o n) -> p o n", o=2),
                                                start=(tt == 0),
                                                stop=(tt == n_vis - 2),
                                                perf_mode=DR,
                                            )

                                if prev is not None:
                                    attv(*prev)
                                prev = (g, ptA, ptB)
                            attv(*prev)

                        # normalize: oT[...] = ops[0:64] * exp(-ln ops[64])
                        for r, ops in ((0, opsA), (1, opsB)):
                            lz = stat.tile([1, SR], F32, tag="lz")
                            nc.scalar.activation(lz[:], ops[64:65, :], AF.Ln)
                            rd = stat.tile([1, SR], F32R, tag="rd")
                            with nc.allow_low_precision(
                                reason="f32r recip bcast"
                            ):
                                nc.scalar.activation(
                                    rd[:], lz[:], AF.Exp, scale=-1.0
                                )
                            rb = pM.tile([P, 512], F32, tag="mm",
                                         name=f"rb{hp}_{r}")
                            nc.tensor.matmul(
                                rb[0:64, :], ones64[:], rd[:],
                                start=True, stop=True,
                            )
                            rbs = stat.tile([64, SR], F32, tag="rbs")
                            nc.vector.tensor_copy(rbs[:], rb[0:64, :])
                            nc.vector.tensor_mul(
                                oT[64 * r:64 * (r + 1),
                                   hp * SR:(hp + 1) * SR],
                                ops[0:64, :], rbs[:],
                            )
                        if hp < NHP - 1:
                            kT, qT = kT_n, qT_n

            # ============ proj + residual -> res; LN2 -> h2T; FFN =======
            oTr = oT[:].rearrange("p (hp s) -> p hp s", hp=NHP)
            wpj = wp_sb[:].rearrange("p (hp c) -> p hp c", hp=NHP)
            with tc.tile_pool(name="post", bufs=1) as post:
                h2T = post.tile([P, NCt * SR], BF16)
                gT = post.tile([P, NF * SR], BF16)
                with (
                    tc.tile_pool(name="pP2", bufs=3, space="PSUM") as pP2,
                    tc.tile_pool(name="pT2", bufs=2, space="PSUM") as pT2,
                ):
                    for m in range(NM):
                        for nh in range(2):
                            ps = pP2.tile([P, 512], F32, tag="mm")
                            for n2 in range(2):
                                for hpp in range(4):
                                    nc.tensor.matmul(
                                        ps[:, n2 * 256:(n2 + 1) * 256],
                                        oTr[:, 2 * hpp:2 * hpp + 2,
                                            m * P:(m + 1) * P],
                                        wpj[:, 2 * hpp:2 * hpp + 2,
                                            nh * 512 + n2 * 256:
                                            nh * 512 + (n2 + 1) * 256],
                                        start=(hpp == 0), stop=False,
                                        perf_mode=(
                                            mybir.MatmulPerfMode.DoubleRow),
                                    )
                            # proj bias via K=1 matmul (ones x bias row)
                            nc.tensor.matmul(
                                ps[:], onescol[:],
                                projb_sb[:, nh * 512:(nh + 1) * 512],
                                start=False, stop=True,
                                skip_group_check=True,
                            )
                            rm = res[:, m * C + nh * 512:
                                     m * C + (nh + 1) * 512]
                            # res += proj_psum / (WS*WS)
                            nc.vector.scalar_tensor_tensor(
                                out=rm, in0=ps[:], scalar=1.0 / (WS * WS),
                                in1=rm, op0=ALU.mult, op1=ALU.add,
                            )
                    rstd, nmean = _ln_group_stats(
                        nc, stat,
                        [res[:, m * C:(m + 1) * C] for m in range(NM)],
                        eps_sb[:],
                    )
                    h2Tj = h2T[:].rearrange("p (j s) -> p j s", j=NCt)
                    for m in range(NM):
                        h2 = wB.tile([P, C], BF16, tag="h2")
                        nc.scalar.activation(
                            h2[:], res[:, m * C:(m + 1) * C], AF.Identity,
                            bias=nmean[:, m:m + 1], scale=rstd[:, m:m + 1],
                        )
                        tp = pT2.tile([P, C], BF16, tag="tp")
                        for j in range(NCt):
                            nc.tensor.transpose(
                                tp[:, j * P:(j + 1) * P],
                                h2[:, j * P:(j + 1) * P], id_sb[:]
                            )
                        nc.vector.tensor_copy(
                            h2Tj[:, :, m * P:(m + 1) * P],
                            tp[:].rearrange("p (j q) -> p j q", j=NCt),
                        )
                        # fold lin2 bias into res for the final add
                        nc.vector.tensor_add(
                            res[:, m * C:(m + 1) * C],
                            res[:, m * C:(m + 1) * C], blin2_sb[:],
                        )

                # ============ FFN1 (gelu) -> gT ============
                l1_t = lin1.rearrange("(n p) f -> n p f", p=P)
                with (
                    tc.tile_pool(name="ffB", bufs=2) as ffB,
                    tc.tile_pool(name="pG", bufs=2, space="PSUM") as pG,
                ):
                    for quar in range(4):
                        l1h = ffB.tile([P, NCt * C], BF16, tag="l1")
                        for j in range(NCt):
                            nc.sync.dma_start(
                                l1h[:, j * C:(j + 1) * C],
                                l1_t[j][:, quar * C:(quar + 1) * C],
                            )
                        for fl in range(NF // 4):
                            ft = quar * (NF // 4) + fl
                            ps = pG.tile([P, 512], F32, tag="mm")
                            for j in range(NCt):
                                nc.tensor.matmul(
                                    ps[:],
                                    l1h[:, j * C + fl * P:
                                        j * C + (fl + 1) * P],
                                    h2T[:, j * SR:(j + 1) * SR],
                                    start=(j == 0), stop=(j == NCt - 1),
                                )
                            nc.scalar.activation(
                                gT[:, ft * SR:(ft + 1) * SR], ps[:], AF.Gelu,
                                bias=blin1_sb[:, ft:ft + 1],
                            )

                # ============ FFN2 + residual -> out ============
                l2_t = lin2.rearrange("(n p) c -> n p c", p=P)
                l2_sb = post.tile([P, NF * C], BF16)
                for ft in range(NF):
                    nc.sync.dma_start(
                        l2_sb[:, ft * C:(ft + 1) * C], l2_t[ft]
                    )
                with (
                    tc.tile_pool(name="ffW", bufs=3) as ffW,
                    tc.tile_pool(name="pF", bufs=2, space="PSUM") as pF,
                ):
                    for nh in range(2):
                        for m in range(NM):
                            fpt = pF.tile([P, 512], F32, tag="ff")
                            for ft in range(NF):
                                nc.tensor.matmul(
                                    fpt[:],
                                    gT[:, ft * SR + m * P:
                                       ft * SR + (m + 1) * P],
                                    l2_sb[:, ft * C + nh * 512:
                                          ft * C + (nh + 1) * 512],
                                    start=(ft == 0), stop=(ft == NF - 1),
                                )
                            o_sb = ffW.tile([P, 512], F32, tag="osb")
                            nc.vector.tensor_add(
                                o_sb[:], fpt[:],
                                res[:, m * C + nh * 512:
                                    m * C + (nh + 1) * 512],
                            )
                            nc.sync.dma_start(
                                out_tiles[m][:, nh * 512:(nh + 1) * 512],
                                o_sb[:],
                            )

    nc.compile()
    return nc


_NC = None


def _get_nc():
    global _NC
    if _NC is None:
        _NC = build_nc()
    return _NC


def kernel(**inputs):
    nc = _get_nc()
    bf = ml_dtypes.bfloat16
    f8 = ml_dtypes.float8_e4m3
    f32 = np.float32

    x = np.asarray(inputs["x"], f32)
    Wq = np.asarray(inputs["Wq"], f32)
    Wk = np.asarray(inputs["Wk"], f32)
    Wv = np.asarray(inputs["Wv"], f32)
    bq = np.asarray(inputs["bq"], f32)
    bv = np.asarray(inputs["bv"], f32)
    proj_w = np.asarray(inputs["proj_w"], f32)
    proj_b = np.asarray(inputs["proj_b"], f32)
    ln1_w = np.asarray(inputs["ln1_w"], f32)
    ln1_b = np.asarray(inputs["ln1_b"], f32)
    ln2_w = np.asarray(inputs["ln2_w"], f32)
    ln2_b = np.asarray(inputs["ln2_b"], f32)
    lin1_w = np.asarray(inputs["lin1_w"], f32)
    lin1_b = np.asarray(inputs["lin1_b"], f32)
    lin2_w = np.asarray(inputs["lin2_w"], f32)
    lin2_b = np.asarray(inputs["lin2_b"], f32)

    # LN1 gamma folded into QKV weights; beta folded into biases.
    # Weights host-scaled x32 for fp8 (0.02-scale weights would land in
    # e4m3's subnormal range); scale compensated via exp-scale / biases.
    Wq_f = Wq * ln1_w[None, :, None] * WS
    Wk_f = Wk * ln1_w[None, :, None] * WS
    Wv_f = Wv * ln1_w[None, :, None] * WS
    bq_eff = (bq + np.einsum("c,hcd->hd", ln1_b, Wq)) * WS
    bv_eff = (bv + np.einsum("c,hcd->hd", ln1_b, Wv)) * WS
    # (bk dropped: softmax shift invariance)
    wq_full = np.ascontiguousarray(
        Wq_f.transpose(1, 0, 2).reshape(C, C)).astype(f8)
    wk_full = np.ascontiguousarray(
        Wk_f.transpose(1, 0, 2).reshape(C, C)).astype(f8)
    wv_full = np.ascontiguousarray(
        Wv_f.transpose(1, 0, 2).reshape(C, C)).astype(f8)
    bq_t = np.ascontiguousarray(bq_eff.reshape(NHP, P).T).astype(f32)
    bv_rowh = bv_eff.reshape(1, C).astype(bf)

    # LN2 gamma folded into lin1; beta into its bias.
    lin1_f = (lin1_w * ln2_w[:, None]).astype(bf)
    blin1_eff = lin1_b + ln2_b @ lin1_w
    blin1_t = np.ascontiguousarray(blin1_eff.reshape(NF, P).T).astype(f32)
    lin2_bf = lin2_w.astype(bf)
    blin2_bc = np.ascontiguousarray(
        np.broadcast_to(lin2_b, (P, C))).astype(f32)

    proj_f8 = (proj_w * WS).astype(f8)
    projb_row = (proj_b * WS * WS).reshape(1, C).astype(bf)
    ident = np.eye(P, dtype=bf)
    ident8 = np.eye(P, dtype=f8)

    in_maps = []
    for c in range(8):
        b, g = divmod(c, 2)
        # my q rows: 64-row blocks {2j+g}, local row 64j+r
        j = np.arange(NCt)
        r = np.arange(64)
        rows = (T - CUT) + 64 * (2 * j[:, None] + g) + r[None, :]
        rows = rows.reshape(-1)
        xq_c = np.ascontiguousarray(x[b][rows]).astype(f32)
        # masks: [128, 4*256]: block i, col 64*jj+rr:
        #   visible iff toff <= 128*jj + 64*g + rr - 128*i
        toff = np.arange(P)[:, None]
        i_b = np.arange(4)[:, None, None]
        jj = np.arange(4)[None, :, None]
        rr = np.arange(64)[None, None, :]
        thr = (128 * jj + 64 * g + rr - 128 * i_b).reshape(1, 1024)
        mask_c = (toff <= thr).astype(f8)
        in_maps.append({
            "x": np.ascontiguousarray(x[b]),
            "xq": xq_c,
            "wq": wq_full, "wk": wk_full, "wv": wv_full,
            "bq": bq_t, "bv_row": bv_rowh,
            "wproj": proj_f8, "projb": projb_row,
            "lin1": lin1_f, "blin1": blin1_t,
            "lin2": lin2_bf, "blin2_bc": blin2_bc,
            "ident": ident, "ident8": ident8,
            "masks": np.ascontiguousarray(mask_c),
        })

    resl = run_bass_kernel_spmd(nc, in_maps, core_ids=list(range(8)))
    out_full = np.empty((B, CUT, C), f32)
    jj = np.arange(NCt)
    rr = np.arange(64)
    for c in range(8):
        b, g = divmod(c, 2)
        rows = (64 * (2 * jj[:, None] + g) + rr[None, :]).reshape(-1)
        out_full[b, rows, :] = resl.results[c]["out"]
    return out_full
